# revision 25
# baseline (speedup 1.0000x reference)
"""Trainium2 Bass kernel for nn_AdaptiveAngleConv.

Reference computes, for each of 5 angles, a bilinear "deformable" 3x3
sampling of x (2,256,64,64) into a (2,256,192,192) image, then a 3x3
VALID conv (stride 1) with a shared weight (256,256,3,3), giving 5
outputs of (2,256,190,190).

Key math: the reference's clipped bilinear sampling is exactly an
UNclipped separable 2x2 stencil with constant per-(angle, n)
coefficients on a zero-padded x — every clipped index lands on a
zero-pad row/col, so the clip never changes a nonzero contribution.
Angles 0/90/180 have integer offsets (pure shifted copies); 45/135 need
a 2-pass (rows then cols) lerp, with the shifted-difference trick so
each pass is one fused scalar_tensor_tensor op per distinct offset.

Sharding: output rows are split across the 8 cores (24 rows each, 8*24
= 192 >= 190). Each core receives a pre-sliced 13-row input slab so the
SPMD graph is identical on every core; no collectives. Per core, the
conv is 18 accumulating fp16 matmuls (9 taps x 2 cin tiles) per 512
flat-pixel chunk, 2 cout tiles, for each of the 10 (angle, batch) jobs.
The 192-wide xo rows are processed flattened; cols 190/191 of each row
are garbage and never stored.

Angle 0 gets a cheaper path: in the mod-3 phase decomposition of the
output, its conv taps collide (g(m) = m//3 + m%3 - 1 maps m=1,3 and
m=2,4 together), so each phase needs only 4-9 collapsed taps (49 total
vs 81) using host-precomputed summed weights, and the matmuls read the
input slab directly with no sampled-image build at all. The two
collapsed jobs bracket the schedule (first for batch 0, last for batch
1) to minimize pipeline head/tail; fp16 keeps the PE at 1 cycle/row
with ~8x better rounding than bf16.
"""

import os
import sys

for _p in ("/opt/trn_rl_repo", "/root/.axon_site/_ro/trn_rl_repo"):
    if os.path.isdir(_p) and _p not in sys.path:
        sys.path.insert(0, _p)

import numpy as np

import concourse.bass as bass
import concourse.mybir as mybir
from concourse import bacc, tile
from concourse.alu_op_type import AluOpType
from concourse.bass_utils import run_bass_kernel_spmd

F32 = mybir.dt.float32
BF16 = mybir.dt.float16  # fp16: same 1 cyc/row PE rate as bf16, 3 more mantissa bits

S2 = 2 ** 0.5
ANGLES = [0, 45, 90, 135, 180]
_OFF = {
    0: ([0.0] * 9, [0.0] * 9),
    1: ([1 - S2, 1 - S2 * 0.5, 1, -S2 * 0.5, 0, S2 * 0.5, -1, S2 * 0.5 - 1, S2 - 1],
        [1, S2 * 0.5, S2 - 1, 1 - S2 * 0.5, 0, S2 * 0.5 - 1, 1 - S2, -S2 * 0.5, -1]),
    2: ([0, 1, 2, -1, 0, 1, -2, -1, 0],
        [2, 1, 0, 1, 0, -1, 0, -1, -2]),
    3: ([1, 1 + S2 * 0.5, 1 + S2, -S2 * 0.5, 0, S2 * 0.5, -1 - S2, -1 - S2 * 0.5, -1],
        [1 + S2, S2 * 0.5, -1, 1 + S2 * 0.5, 0, -1 - S2 * 0.5, 1, -S2 * 0.5, 1 + S2]),
    4: ([2, 2, 2, 0, 0, 0, -2, -2, -2],
        [2, 0, -2, 2, 0, -2, 2, 0, -2]),
}

NCORES = 8
NR = 24            # output rows per core (8*24 = 192, rows 190/191 dropped)
SLAB_ROWS = 13     # input rows a core needs: hi in [8k-2, 8k+10]
SLAB_COLS = 70     # data cols -2..67
XO_ROWS = 26       # NR + 2 halo rows of the sampled image
XO_F = XO_ROWS * 192


def _tables():
    """Per angle: list of (n, r, s, Ax, fx, Ay, fy) in f32 semantics."""
    rng = np.arange(-1, 2)
    pnx, pny = np.meshgrid(rng, rng, indexing="ij")
    pnx = pnx.reshape(-1).astype(np.float32)
    pny = pny.reshape(-1).astype(np.float32)
    out = {}
    for a in ANGLES:
        ox, oy = _OFF[a // 45]
        dx = pnx + np.array(ox, dtype=np.float32)
        dy = pny + np.array(oy, dtype=np.float32)
        rows = []
        for n in range(9):
            Ax = int(np.floor(dx[n]))
            Ay = int(np.floor(dy[n]))
            fx = float(np.float32(dx[n] - Ax))
            fy = float(np.float32(dy[n] - Ay))
            rows.append((n, n // 3, n % 3, Ax, fx, Ay, fy))
        out[a] = rows
    return out


TABLES = _tables()
# distinct fractional row offsets shared by the 45/135 pair
LERP_DS = sorted({(t[3], t[4]) for a in (45, 135) for t in TABLES[a]})

# Angle-0 phase-collapsed conv: output phase rho uses row taps di with the
# listed combo of original kernel rows (g(m)=m//3+m%3-1 collides for m=1,3
# and m=2,4). Combo indices into the host-precomputed sums: 0,1,2 = single
# ki, 3 = ki0+ki2. Same structure for columns. 49 taps/phase-grid vs 81.
ROW_COMBOS = [(0,), (1,), (2,), (0, 2)]
PHROWS = {0: [(-1, 0), (0, 1), (1, 2)],
          1: [(0, 3), (1, 1)],
          2: [(1, 3), (0, 1)]}


def build_graph():
    nc = bacc.Bacc()
    xs = nc.declare_dram_parameter("xs", [2, 2, 128, SLAB_ROWS * SLAB_COLS], BF16, False)
    wt = nc.declare_dram_parameter("wt", [2, 128, 9 * 2 * 128], BF16, False)
    wc = nc.declare_dram_parameter("wc", [2, 128, 16 * 2 * 128], BF16, False)
    out = nc.declare_dram_parameter("out", [5, 2, 2, 128, NR, 190], F32, True)

    with tile.TileContext(nc) as tc:
        with (
            tc.tile_pool(name="const", bufs=1) as constp,
            tc.tile_pool(name="xop", bufs=2) as xop,
            tc.tile_pool(name="rcp", bufs=1) as rcp,
            tc.tile_pool(name="stg", bufs=2) as stgp,
            tc.tile_pool(name="ps", bufs=8, space="PSUM") as psp,
        ):
            # HAM warm-up: dependency-free matmuls on an uninitialized tile
            # keep the PE busy during the input-DMA window so the clock gate
            # is already at 8/8 when the first real matmul issues. Results
            # land in a scratch PSUM bank and are never read.
            warm = constp.tile([128, 640], BF16, name="warm", tag="warm")
            nc.gpsimd.memset(warm[:], 0.0)
            wps = psp.tile([128, 512], F32, name="wps", tag="ps")
            for _ in range(16):
                nc.tensor.matmul(wps[:], warm[:, :128], warm[:, 128:640],
                                 start=True, stop=True)

            # DMA order matters for the head: the first job (collapsed
            # angle-0, batch 0) needs slabs + wc only; wt is needed two
            # jobs later.
            slab = {}

            def load_slab(b):
                for ct in range(2):
                    s = constp.tile([128, SLAB_ROWS * SLAB_COLS], BF16,
                                    name=f"slab{b}{ct}", tag=f"slab{b}{ct}")
                    nc.sync.dma_start(s[:], xs[b, ct])
                    slab[(b, ct)] = s

            load_slab(0)
            # wc is ot-major; load the ot=0 half first so the first job's
            # first matmuls only wait on half the collapsed-weight bytes
            # (Tile tracks subtile deps within the tile).
            wc_sb = []
            for ct in range(2):
                wctile = constp.tile([128, 16 * 2 * 128], BF16, name=f"wc{ct}",
                                     tag=f"wc{ct}")
                nc.sync.dma_start(wctile[:, :2048], wc[ct][:, :2048])
                wc_sb.append(wctile)
            for ct in range(2):
                nc.sync.dma_start(wc_sb[ct][:, 2048:], wc[ct][:, 2048:])
            load_slab(1)
            w_sb = []
            for ct in range(2):
                wtile = constp.tile([128, 9 * 2 * 128], BF16, name=f"w{ct}", tag=f"w{ct}")
                nc.sync.dma_start(wtile[:], wt[ct])
                w_sb.append(wtile)

            def slab3(b, ct):
                return slab[(b, ct)].rearrange("p (r c) -> p r c", c=SLAB_COLS)

            def build_xo_int(a, b):
                """xo tiles for an integer-offset angle via strided copies."""
                xo = []
                for ct in range(2):
                    t = xop.tile([128, XO_F + 16], BF16, name=f"xo{ct}", tag=f"xo{ct}")
                    v = t[:, :XO_F].rearrange("p (r c) -> p r c", c=192)
                    sv = slab3(b, ct)
                    for (n, r, s, Ax, fx, Ay, fy) in TABLES[a]:
                        nrow = 9 if r < 2 else 8
                        src = sv[:, 2 + Ax : 2 + Ax + nrow, 2 + Ay : 66 + Ay]
                        nc.vector.tensor_copy(v[:, r::3, s::3], src)
                    xo.append(t)
                return xo

            def build_lerp_rc(b):
                """Shared row-lerp R_d and col-diff C_d tiles for 45+135."""
                R = {}
                C = {}
                for ct in range(2):
                    sv = slab3(b, ct)
                    dr = rcp.tile([128, 12 * SLAB_COLS], BF16,
                                  name=f"dr{ct}", tag=f"dr{ct}")
                    drv = dr.rearrange("p (r c) -> p r c", c=SLAB_COLS)
                    nc.vector.tensor_tensor(drv, sv[:, 1:13, :], sv[:, 0:12, :],
                                            AluOpType.subtract)
                    for di, (Ax, fx) in enumerate(LERP_DS):
                        if fx == 0.0:
                            rv = sv[:, 2 + Ax : 11 + Ax, :]
                        else:
                            rt = rcp.tile([128, 9 * SLAB_COLS], BF16,
                                          name=f"r{ct}_{di}", tag=f"r{ct}_{di}")
                            rv = rt.rearrange("p (r c) -> p r c", c=SLAB_COLS)
                            nc.vector.scalar_tensor_tensor(
                                rv, drv[:, 2 + Ax : 11 + Ax, :], fx,
                                sv[:, 2 + Ax : 11 + Ax, :],
                                AluOpType.mult, AluOpType.add)
                        ctile = rcp.tile([128, 9 * SLAB_COLS], BF16,
                                         name=f"c{ct}_{di}", tag=f"c{ct}_{di}")
                        cv = ctile.rearrange("p (r c) -> p r c", c=SLAB_COLS)
                        nc.vector.tensor_tensor(cv[:, :, 0:69], rv[:, :, 1:70],
                                                rv[:, :, 0:69], AluOpType.subtract)
                        R[(ct, Ax, fx)] = rv
                        C[(ct, Ax, fx)] = cv
                return R, C

            def build_xo_lerp(a, b, R, C):
                xo = []
                for ct in range(2):
                    t = xop.tile([128, XO_F + 16], BF16, name=f"xo{ct}", tag=f"xo{ct}")
                    v = t[:, :XO_F].rearrange("p (r c) -> p r c", c=192)
                    for (n, r, s, Ax, fx, Ay, fy) in TABLES[a]:
                        nrow = 9 if r < 2 else 8
                        rv = R[(ct, Ax, fx)][:, :nrow, 2 + Ay : 66 + Ay]
                        if fy == 0.0:
                            nc.vector.tensor_copy(v[:, r::3, s::3], rv)
                        else:
                            cv = C[(ct, Ax, fx)][:, :nrow, 2 + Ay : 66 + Ay]
                            nc.vector.scalar_tensor_tensor(
                                v[:, r::3, s::3], cv, fy, rv,
                                AluOpType.mult, AluOpType.add)
                    xo.append(t)
                return xo

            def conv_job_angle0(ai, b):
                """Phase-collapsed conv reading the slab directly (no xo).
                Groups by output row phase rho so each rho's rows can DMA
                out as soon as its three column phases are evacuated."""
                for ot in range(2):
                    stg = stgp.tile([128, NR * 192], F32, name="stg", tag="stg")
                    stgv = stg.rearrange("p (r c) -> p r c", c=192)
                    for rho in range(3):
                        ps = {sig: psp.tile([128, 512], F32, name=f"ps{sig}",
                                            tag="ps") for sig in range(3)}
                        for sig in range(3):
                            taps = [(di, dj, ri * 4 + ci)
                                    for (di, ri) in PHROWS[rho]
                                    for (dj, ci) in PHROWS[sig]]
                            nmm = len(taps) * 2
                            i = 0
                            for (di, dj, cb) in taps:
                                for ct in range(2):
                                    sv = slab3(b, ct)
                                    w_ap = wc_sb[ct][:, (ot * 16 + cb) * 128 :
                                                     (ot * 16 + cb + 1) * 128]
                                    nc.tensor.matmul(
                                        ps[sig][:], w_ap,
                                        sv[:, di + 2 : di + 10, dj + 2 : dj + 66],
                                        start=(i == 0), stop=(i == nmm - 1))
                                    i += 1
                        for sig in range(3):
                            psv = ps[sig].rearrange("p (r c) -> p r c", c=64)
                            nc.scalar.copy(stgv[:, rho::3, sig::3], psv)
                        nc.sync.dma_start(out[ai, b, ot, :, rho::3, :],
                                          stgv[:, rho : NR : 3, :190])

            def conv_job(ai, b, xo):
                for ot in range(2):
                    stg = stgp.tile([128, NR * 192], F32, name="stg", tag="stg")
                    for chunks in ((0, 1, 2, 3), (4, 5, 6, 7), (8,)):
                        ps = {ch: psp.tile([128, 512], F32, name=f"ps{ch}", tag="ps")
                              for ch in chunks}
                        for kk in range(9):
                            ki, kj = divmod(kk, 3)
                            for ct in range(2):
                                w_ap = w_sb[ct][:, (kk * 2 + ot) * 128 :
                                                (kk * 2 + ot + 1) * 128]
                                for ch in chunks:
                                    off = ch * 512 + ki * 192 + kj
                                    nc.tensor.matmul(
                                        ps[ch][:], w_ap, xo[ct][:, off : off + 512],
                                        start=(kk == 0 and ct == 0),
                                        stop=(kk == 8 and ct == 1))
                        for ch in chunks:
                            nc.scalar.copy(stg[:, ch * 512 : (ch + 1) * 512],
                                           ps[ch][:])
                    sv = stg.rearrange("p (r c) -> p r c", c=192)
                    nc.sync.dma_start(out[ai, b, ot], sv[:, :NR, :190])

            # angle 0 (no xo build) first for b=0 so the PE starts on the
            # slab DMA alone, and last for b=1 so the tail is the staggered
            # per-rho DMAs of the collapsed job.
            conv_job_angle0(0, 0)
            for b in range(2):
                R, C = build_lerp_rc(b)
                xo = build_xo_lerp(45, b, R, C)
                conv_job(1, b, xo)
                xo = build_xo_lerp(135, b, R, C)
                conv_job(3, b, xo)
                xo = build_xo_int(90, b)
                conv_job(2, b, xo)
                xo = build_xo_int(180, b)
                conv_job(4, b, xo)
            conv_job_angle0(0, 1)

    nc.compile()
    return nc


_GRAPH = None


def _graph():
    global _GRAPH
    if _GRAPH is None:
        _GRAPH = build_graph()
    return _GRAPH


def prep_inputs(x, weight):
    x = np.asarray(x, dtype=np.float32)
    weight = np.asarray(weight, dtype=np.float32)
    # pad data rows -2..66, cols -2..67
    xp = np.pad(x, ((0, 0), (0, 0), (2, 3), (2, 4))).astype(np.float16)
    xs_cores = []
    for k in range(NCORES):
        sl = xp[:, :, 8 * k : 8 * k + SLAB_ROWS, :]          # [2,256,13,70]
        sl = sl.reshape(2, 2, 128, SLAB_ROWS * SLAB_COLS)
        xs_cores.append(np.ascontiguousarray(sl))
    w6 = weight.reshape(2, 128, 2, 128, 3, 3)                 # [ot,o,ct,c,ki,kj]
    w = w6.transpose(2, 3, 4, 5, 0, 1).reshape(2, 128, 9 * 2 * 128)
    w = np.ascontiguousarray(w.astype(np.float16))
    combos = []
    for R in ROW_COMBOS:
        for C in ROW_COMBOS:
            combos.append(w6[..., list(R), :][..., list(C)].sum(axis=(-1, -2)))
    wcarr = np.stack(combos, axis=0)                          # [16,ot,o,ct,c]
    wcarr = wcarr.transpose(3, 4, 1, 0, 2).reshape(2, 128, 2 * 16 * 128)
    wcarr = np.ascontiguousarray(wcarr.astype(np.float16))
    return xs_cores, w, wcarr


def assemble(results):
    full = np.empty((5, 2, 256, NCORES * NR, 190), np.float32)
    for k in range(NCORES):
        o = results[k]["out"]                                 # [5,2,2,128,24,190]
        o = o.reshape(5, 2, 256, NR, 190)
        full[:, :, :, NR * k : NR * (k + 1), :] = o
    full = full[:, :, :, :190, :]
    return tuple(np.ascontiguousarray(full[i]) for i in range(5))


def run(x, weight, trace=False, **trace_kw):
    xs_cores, w, wcarr = prep_inputs(x, weight)
    nc = _graph()
    in_maps = [{"xs": xs_cores[k], "wt": w, "wc": wcarr} for k in range(NCORES)]
    res = run_bass_kernel_spmd(nc, in_maps, core_ids=list(range(NCORES)),
                               trace=trace, **trace_kw)
    return assemble(res.results), res


def kernel(x, weight):
    return run(x, weight)[0]
